# revision 20
# baseline (speedup 1.0000x reference)
"""EnhancedMultiHeadAttention on 8 Trainium2 NeuronCores.

Sharding: 8 cores = 2 batches x 4 head-groups (4 heads / 256 columns each).
Each core computes Q/K/V projections for its (batch, head-group), attention
in a fully transposed layout (scores^T = K @ Q^T, so softmax denominators
come from an extra ones-column in V and no PE transposes are needed), and a
partial output projection with its slice of wo rows.  The host sums the 4
group partials per batch and adds bo.

All matmuls run as float32r (FP22 multiply, FP32 accumulate).
"""

import sys

for _p in ("/opt/trn_rl_repo", "/root/.axon_site/_ro/trn_rl_repo"):
    if _p not in sys.path:
        sys.path.append(_p)

import numpy as np

import concourse.bass as bass
import concourse.mybir as mybir
import concourse.tile as tile
from concourse import bacc
from concourse.bass_utils import run_bass_kernel_spmd
from concourse.masks import make_identity

F32 = mybir.dt.float32
F32R = mybir.dt.float32r

B, S, D = 2, 2048, 1024
H, DEPTH = 16, 64
NCORES = 8
GROUPS = 4                  # head-groups per batch
HC = H // GROUPS            # heads per core = 4
C = HC * DEPTH              # columns per core = 256
NPAIR = HC // 2             # head pairs per core = 2
DT = D // 128               # 8 d-tiles
TT = S // 128               # 16 token tiles
QB = S // 512               # 4 q blocks
KT = S // 128               # 16 k tiles
SCALE = 0.125               # 1/sqrt(DEPTH)


def _r(ap):
    return ap


def build_nc():
    nc = bacc.Bacc(None, target_bir_lowering=False)

    xq = nc.dram_tensor("xq", [D, S], F32R, kind="ExternalInput")
    xk = nc.dram_tensor("xk", [D, S], F32R, kind="ExternalInput")
    xv = nc.dram_tensor("xv", [D, S], F32R, kind="ExternalInput")
    wq = nc.dram_tensor("wq", [D, C], F32R, kind="ExternalInput")
    wk = nc.dram_tensor("wk", [D, C], F32R, kind="ExternalInput")
    wv = nc.dram_tensor("wv", [D, C], F32R, kind="ExternalInput")
    wo = nc.dram_tensor("wo", [C, D], F32R, kind="ExternalInput")
    bq = nc.dram_tensor("bq", [C], F32, kind="ExternalInput")
    bk = nc.dram_tensor("bk", [C], F32, kind="ExternalInput")
    bv = nc.dram_tensor("bv", [C], F32, kind="ExternalInput")
    expb = nc.dram_tensor("expb", [S], F32, kind="ExternalInput")
    out = nc.dram_tensor("out", [S, D], F32, kind="ExternalOutput")

    with tile.TileContext(nc) as tc, nc.allow_low_precision(
        reason="float32r tiles are 4-byte storage; accumulation stays fp32 in PSUM"
    ):
        with (
            tc.tile_pool(name="wpool", bufs=1) as wp,
            tc.tile_pool(name="qk", bufs=1) as qkp,
            tc.tile_pool(name="vsb", bufs=1) as vp,
            tc.tile_pool(name="ctx", bufs=1) as cxp,
        ):
            # ---- resident weights / biases ----
            wq_sb = wp.tile([128, DT, C], F32R)
            wk_sb = wp.tile([128, DT, C], F32R)
            wv_sb = wp.tile([128, DT, C], F32R)
            nc.sync.dma_start(wv_sb[:], wv.rearrange("(dt p) c -> p dt c", p=128))
            nc.sync.dma_start(wk_sb[:], wk.rearrange("(dt p) c -> p dt c", p=128))
            nc.sync.dma_start(wq_sb[:], wq.rearrange("(dt p) c -> p dt c", p=128))
            wo_sb = wp.tile([128, 2, D], F32R)
            nc.sync.dma_start(wo_sb[:], wo.rearrange("(ct p) n -> p ct n", p=128))
            bq_sb = wp.tile([128, 2], F32)
            bk_sb = wp.tile([128, 2], F32)
            nc.sync.dma_start(bq_sb[:], bq.rearrange("(ct p) -> p ct", p=128))
            nc.sync.dma_start(bk_sb[:], bk.rearrange("(ct p) -> p ct", p=128))
            bv_sb = wp.tile([128, 2], F32)
            nc.sync.dma_start(bv_sb[:], bv.rearrange("(ct p) -> p ct", p=128))
            ident = wp.tile([128, 128], F32)
            make_identity(nc, ident[:])
            expb_sb = wp.tile([128, TT], F32)
            nc.sync.dma_start(expb_sb[:], expb.rearrange("(tt p) -> p tt", p=128))

            # ---- persistent activations ----
            qT = [qkp.tile([128, S], F32R, tag=f"qT{i}", name=f"qT{i}") for i in range(NPAIR)]
            kT = [qkp.tile([128, S], F32R, tag=f"kT{i}", name=f"kT{i}") for i in range(NPAIR)]
            # V': per token tile [128, HC, 65]; [:, h, :64] = (V + bv)*expB,
            # [:, h, 64] = expB (softmax denominator column)
            vs = [vp.tile([128, HC, 65], F32R, tag=f"vs{t}", name=f"vs{t}") for t in range(TT)]
            ctx = [cxp.tile([128, S], F32R, tag=f"ctx{i}", name=f"ctx{i}") for i in range(NPAIR)]

            # ================= projections =================
            with (
                tc.tile_pool(name="xs", bufs=6) as xsp,
                tc.tile_pool(name="vT", bufs=1) as vTp,
                tc.tile_pool(name="pp", bufs=8, space="PSUM") as pp,
            ):
                vT = [vTp.tile([128, S], F32, tag=f"vT{i}", name=f"vT{i}") for i in range(NPAIR)]
                # ---- Q^T / K^T / V^T projections, all streaming over d ----
                for x_dram, w_sb, b_sb, dst, ddt in (
                    (xv, wv_sb, bv_sb, vT, F32),
                    (xk, wk_sb, bk_sb, kT, F32R),
                    (xq, wq_sb, bq_sb, qT, F32R),
                ):
                    xin = x_dram.rearrange("(dt p) t -> dt p t", p=128)
                    ps = {}
                    for dt in range(DT):
                        xt = xsp.tile([128, S], F32R, tag="xt", name="xt")
                        nc.sync.dma_start(xt[:], xin[dt])
                        for ct in range(2):
                            for tb in range(QB):
                                if dt == 0:
                                    ps[ct, tb] = pp.tile([128, 512], F32, tag="pp", name=f"ps{ct}_{tb}")
                                nc.tensor.matmul(
                                    ps[ct, tb][:],
                                    _r(w_sb[:, dt, ct * 128 : (ct + 1) * 128]),
                                    _r(xt[:, tb * 512 : (tb + 1) * 512]),
                                    start=(dt == 0),
                                    stop=(dt == DT - 1),
                                )
                    for ct in range(2):
                        for tb in range(QB):
                            nc.scalar.activation(
                                dst[ct][:, tb * 512 : (tb + 1) * 512],
                                ps[ct, tb][:],
                                mybir.ActivationFunctionType.Identity,
                                bias=b_sb[:, ct : ct + 1],
                                scale=1.0,
                            )
                    if x_dram is xv:
                        # transpose V^T -> V' right away: runs while xk/xq stream in
                        for tt in range(TT):
                            for pr in range(NPAIR):
                                tp = pp.tile([128, 128], F32, tag="pp", name="tp")
                                nc.tensor.transpose(
                                    tp[:], vT[pr][:, tt * 128 : (tt + 1) * 128], ident[:]
                                )
                                dst3 = vs[tt][:, pr * 2 : pr * 2 + 2, 0:64]
                                src3 = tp[:].rearrange("p (h d) -> p h d", h=2)
                                nc.vector.tensor_scalar_mul(
                                    dst3, src3, expb_sb[:, tt : tt + 1]
                                )
                            for h in range(HC):
                                nc.vector.tensor_copy(
                                    vs[tt][:, h, 64:65], expb_sb[:, tt : tt + 1]
                                )

            # ========== attention + interleaved output projection ==========
            # Software-pipelined: AV for k-tile g is emitted after QK for
            # g+1, so the PE never sits idle waiting for the exp of its own
            # group.  Normalization has no PE ops (denominator broadcast
            # goes through a DRAM bounce), so it never stalls the PE queue.
            with (
                tc.tile_pool(name="sps", bufs=2, space="PSUM") as sps,
                tc.tile_pool(name="wps", bufs=4, space="PSUM") as wps,
                tc.tile_pool(name="pex", bufs=6) as pex,
                tc.tile_pool(name="nrm", bufs=2) as nrm,
                tc.tile_pool(name="osb", bufs=2) as osb,
                tc.tile_pool(name="dsc", bufs=4, space="DRAM") as dsc,
            ):
                oout = out.rearrange("(qt p) n -> qt p n", p=128)
                units = [(qb, pr) for qb in range(QB) for pr in range(NPAIR)]
                flat = [(qb, pr, kt) for qb, pr in units for kt in range(KT)]
                pending = {}  # (qb, pr, kt) -> (st_pe_tile)
                avs = {}      # (qb, pr) -> [av0, av1]

                def emit_qk(qb, pr, kt):
                    qsl = slice(qb * 512, (qb + 1) * 512)
                    ksl = slice(kt * 128, (kt + 1) * 128)
                    st = sps.tile([128, 1024], F32, tag="s", name="st")
                    for hh in range(2):
                        psl = slice(hh * 64, (hh + 1) * 64)
                        nc.tensor.matmul(
                            st[:, hh * 512 : (hh + 1) * 512],
                            _r(kT[pr][psl, ksl]),
                            _r(qT[pr][psl, qsl]),
                        )
                    pe = pex.tile([128, 1024], F32R, tag="pe", name="pe")
                    nc.scalar.activation(
                        pe[:], st[:], mybir.ActivationFunctionType.Exp, scale=SCALE
                    )
                    pending[qb, pr, kt] = pe

                def emit_av(qb, pr, kt):
                    pe = pending.pop((qb, pr, kt))
                    if kt == 0:
                        avs[qb, pr] = [
                            wps.tile([65, 512], F32, tag="w", name="av") for _ in range(2)
                        ]
                    av = avs[qb, pr]
                    for hh in range(2):
                        nc.tensor.matmul(
                            av[hh][:],
                            _r(vs[kt][:, pr * 2 + hh, :]),
                            _r(pe[:, hh * 512 : (hh + 1) * 512]),
                            start=(kt == 0),
                            stop=(kt == KT - 1),
                        )
                    if kt == KT - 1:
                        emit_norm(qb, pr)

                def emit_norm(qb, pr):
                    qsl = slice(qb * 512, (qb + 1) * 512)
                    av = avs.pop((qb, pr))
                    for hh in range(2):
                        rec = nrm.tile([65, 512], F32R, tag="rec", name="rec")
                        nc.vector.reciprocal(rec[:], av[hh][:])
                        dr = dsc.tile([1, 512], F32R, tag="dr", name="dr")
                        nc.sync.dma_start(dr[:], rec[64:65, :])
                        dr_ap = dr[:]
                        bcast = bass.AP(
                            tensor=dr_ap.tensor, offset=dr_ap.offset,
                            ap=[[0, 64]] + [list(a) for a in dr_ap.ap[1:]],
                        )
                        bcs = nrm.tile([64, 512], F32R, tag="bcs", name="bcs")
                        nc.sync.dma_start(bcs[:], bcast)
                        if hh == 0:
                            nc.vector.tensor_mul(
                                ctx[pr][0:64, qsl], av[hh][0:64, :], bcs[:]
                            )
                        else:
                            tmp = nrm.tile([64, 512], F32R, tag="tmp", name="tmp")
                            nc.vector.tensor_mul(tmp[:], av[hh][0:64, :], bcs[:])
                            nc.sync.dma_start(ctx[pr][64:128, qsl], tmp[:])

                ots = {}

                def emit_outproj_chain(qb, j):
                    qt = qb * 4 + j // 2
                    n = j % 2
                    if n == 0:
                        ots[qt] = osb.tile([128, D], F32, tag="ot", name="ot")
                    ot = ots[qt]
                    qts = slice(qt * 128, (qt + 1) * 128)
                    po = wps.tile([128, 512], F32, tag="w", name="po")
                    for ct in range(2):
                        nc.tensor.matmul(
                            po[:],
                            _r(ctx[ct][:, qts]),
                            _r(wo_sb[:, ct, n * 512 : (n + 1) * 512]),
                            start=(ct == 0),
                            stop=(ct == 1),
                        )
                    nc.vector.tensor_copy(ot[:, n * 512 : (n + 1) * 512], po[:])
                    if n == 1:
                        nc.sync.dma_start(oout[qt], ots.pop(qt)[:])

                SKEW = 2
                OPDELAY = 24  # flat positions after a qb's last AV (ctx is ready)
                OPSPACE = 1   # positions between successive out-proj chains
                op_sched = {}
                for qb in range(QB - 1):
                    last_av_pos = (qb * NPAIR + NPAIR) * KT - 1 + SKEW
                    for j in range(8):
                        op_sched[min(last_av_pos + OPDELAY + j * OPSPACE, len(flat) - 1 - (7 - j))] = (qb, j)
                for i, (qb, pr, kt) in enumerate(flat):
                    emit_qk(qb, pr, kt)
                    if i >= SKEW:
                        emit_av(*flat[i - SKEW])
                    if i in op_sched:
                        emit_outproj_chain(*op_sched[i])
                for i in range(len(flat) - SKEW, len(flat)):
                    emit_av(*flat[i])
                for j in range(8):
                    emit_outproj_chain(QB - 1, j)

    nc.finalize()
    return nc


_NC = None


def _get_nc():
    global _NC
    if _NC is None:
        _NC = build_nc()
    return _NC


def make_in_maps(query, key, value, temporal_bias, wq, bq, wk, bk, wv, bv, wo, bo):
    f = np.float32
    xt = {}
    for b in range(B):
        xt["q", b] = np.ascontiguousarray(np.asarray(query[b], f).T)
        xt["k", b] = np.ascontiguousarray(np.asarray(key[b], f).T)
        xt["v", b] = np.ascontiguousarray(np.asarray(value[b], f).T)
    expb = np.exp(np.asarray(temporal_bias, f))
    in_maps = []
    for core in range(NCORES):
        b, g = divmod(core, GROUPS)
        cs = slice(g * C, (g + 1) * C)
        in_maps.append({
            "xq": xt["q", b],
            "xk": xt["k", b],
            "xv": xt["v", b],
            "wq": np.ascontiguousarray(np.asarray(wq, f)[:, cs]),
            "wk": np.ascontiguousarray(np.asarray(wk, f)[:, cs]),
            "wv": np.ascontiguousarray(np.asarray(wv, f)[:, cs]),
            "wo": np.ascontiguousarray(np.asarray(wo, f)[cs, :]),
            "bq": np.ascontiguousarray(np.asarray(bq, f)[cs]),
            "bk": np.ascontiguousarray(np.asarray(bk, f)[cs]),
            "bv": np.ascontiguousarray(np.asarray(bv, f)[cs]),
            "expb": np.ascontiguousarray(expb[b]),
        })
    return in_maps


def gather(results, bo):
    bo = np.asarray(bo, np.float32)
    out = np.zeros((B, S, D), np.float32)
    for core in range(NCORES):
        b = core // GROUPS
        out[b] += results[core]["out"]
    out += bo[None, None, :]
    return out


def kernel(query, key, value, temporal_bias, wq, bq, wk, bk, wv, bv, wo, bo,
           _trace=False):
    nc = _get_nc()
    in_maps = make_in_maps(query, key, value, temporal_bias,
                           wq, bq, wk, bk, wv, bv, wo, bo)
    res = run_bass_kernel_spmd(nc, in_maps, list(range(NCORES)), trace=_trace)
    out = gather(res.results, bo)
    if _trace:
        return out, res
    return out
